# revision 35
# baseline (speedup 1.0000x reference)
"""Causal self-attention (B=4, T=2048, C=768, H=12) on 8 TRN2 NeuronCores.

Sharding: data-parallel over batch (4) x tensor-parallel over heads (2 groups
of 6).  Core c handles batch c//2, head-group c%2.  Each core computes its
QKV projection slice, causal flash-attention for its 6 heads, and a partial
output projection; the host sums the two head-group partials per batch and
adds b_proj.

On-device layout (per core):
  xt    [C, T]   x[b] transposed (host-side) so features sit on partitions.
  qkT   [2*384, T] q^T | k^T feature-major, computed as w_qk^T-chunk @ x^T
        (w_qk columns are pre-shuffled host-side into FC_ORDER so the
        blocks needed first arrive in the leading DMA).
  v1    [T, 6, 128] v natural (token-major) with a 64-wide ones block per
        head: the PV matmul (time is moving-dim bound, the extra stationary
        columns are free) then emits the softmax denominator already
        replicated across 64 partitions, so normalization is just
        reciprocal + multiply on VectorE -- no cross-partition broadcast.
  S^T   computed per 128-row j-tile as k @ q^T so the softmax reduction
        (over j) lands on the partition axis and is folded into the PV
        matmul; exp() runs on ScalarE straight out of PSUM; the causal
        triangle is an affine_select on GpSimd; fully-masked columns are
        never computed or exp'ed.
  yT    [384, T] normalized attention output, feature-major, feeding the
        output projection with w_p natural.
All tensors stream as bf16 (fp32 PSUM accumulation); emission order
hand-interleaves the QKV projection, attention and output projection so
TensorE and ScalarE (exp is ~115us of intrinsic work) overlap; measured
~190-230us/core on HW, output rel err ~3e-3 vs the fp32 reference.
"""
import sys

try:
    import concourse  # noqa: F401
except ImportError:
    sys.path.insert(0, "/opt/trn_rl_repo")

import numpy as np
import concourse.bacc as bacc
import concourse.mybir as mybir
import concourse.tile as tile
from concourse.bass_utils import run_bass_kernel_spmd

f32 = mybir.dt.float32
bf16 = mybir.dt.bfloat16
IN_DT = bf16     # streamed inputs (xt, w_qk, w_v, w_p) + yT
Exp = mybir.ActivationFunctionType.Exp

B, T, C, H = 4, 2048, 768, 12
FC_ORDER = [0, 3, 1, 4, 2, 5]   # host lays w_qk/b_qk columns out in this
FC_POS = {fc: i for i, fc in enumerate(FC_ORDER)}  # feature-chunk order
HD = 64          # head dim
GW = 384         # head-group width (6 heads)
NCC = C // 128   # 6 contraction chunks
SCALE = HD ** -0.5


def _emit(tc, xt, w_qk, w_v, b_qk, b_v, w_p, out, n_reps=1):
    nc = tc.nc

    with tc.tile_pool(name="const", bufs=1) as const, \
         tc.tile_pool(name="qkv", bufs=1) as qkv, \
         tc.tile_pool(name="psp", bufs=2, space="PSUM") as psp, \
         tc.tile_pool(name="pog", bufs=4, space="PSUM") as pog, \
         tc.tile_pool(name="ptp", bufs=16) as ptp, \
         tc.tile_pool(name="nrm", bufs=6) as nrm, \
         tc.tile_pool(name="ob", bufs=5) as ob:
        # ---- constants (tiles only; DMAs issued after the phase-1 bulk
        # loads so their queue triggers don't delay the first matmuls)
        bqk_all = const.tile([128, 6], f32, name="bqk")
        bqk_sb = [bqk_all[:, fc:fc + 1] for fc in range(6)]
        bv_sb = const.tile([128, GW], f32, name="bv")
        ones6 = const.tile([128, 6], f32, name="ones6")
        nc.vector.memset(ones6, 1.0)
        wp_all = const.tile([128, 3, C], IN_DT, name="wp")
        wp_sb = [wp_all[:, fc, :] for fc in range(3)]

        def load_consts():
            nc.sync.dma_start(
                out=bqk_all, in_=b_qk[:].rearrange("(fc p) -> p fc", p=128))
            nc.sync.dma_start(
                out=bv_sb,
                in_=b_v[:][None, :].partition_broadcast(128).opt(keep_dims={0}))
            nc.sync.dma_start(
                out=wp_all, in_=w_p[:, :].rearrange("(fc p) n -> p fc n", p=128))

        # ---- persistent per-rep tensors
        qkT = [qkv.tile([128, T], IN_DT, name=f"qkT{fc}") for fc in range(6)]
        v1 = [qkv.tile([128, 6, 128], IN_DT, name=f"v1_{tt}") for tt in range(16)]
        yT = [qkv.tile([128, T], IN_DT, name=f"yT{fc}") for fc in range(3)]

        for _ in range(n_reps):
            # ================= phase 1: QKV projection =================
            # pog slots are shared with attention PV accumulators and the
            # projection so phases overlap without a PSUM release barrier.
            with tc.tile_pool(name="xw", bufs=1) as xw:
                wqk_all = xw.tile([128, 6, 2 * GW], IN_DT, name="wqk")
                nc.sync.dma_start(
                    out=wqk_all[:, :, 0:256],
                    in_=w_qk[:, 0:256].rearrange("(cc p) f -> p cc f", p=128))
                wqk_sb = [wqk_all[:, cc, :] for cc in range(6)]
                xt_sb = [xw.tile([128, T], IN_DT, name=f"xt{cc}") for cc in range(6)]
                for cc in range(6):
                    nc.scalar.dma_start(out=xt_sb[cc][:, 0:1024],
                                        in_=xt[128 * cc:128 * (cc + 1), 0:1024])
                nc.sync.dma_start(
                    out=wqk_all[:, :, 256:2 * GW],
                    in_=w_qk[:, 256:2 * GW].rearrange("(cc p) f -> p cc f", p=128))
                for cc in range(6):
                    nc.scalar.dma_start(out=xt_sb[cc][:, 1024:T],
                                        in_=xt[128 * cc:128 * (cc + 1), 1024:T])
                wv_all = xw.tile([128, 6, GW], IN_DT, name="wv")
                nc.sync.dma_start(
                    out=wv_all, in_=w_v[:, :].rearrange("(cc p) f -> p cc f", p=128))
                wv_sb = [wv_all[:, cc, :] for cc in range(6)]
                load_consts()

                def qk_chunk(fc):
                    pos = FC_POS[fc]
                    for t4 in range(4):
                        pq = pog.tile([128, 512], f32, name="po")
                        for cc in range(6):
                            nc.tensor.matmul(
                                pq, wqk_sb[cc][:, 128 * pos:128 * (pos + 1)],
                                xt_sb[cc][:, 512 * t4:512 * (t4 + 1)],
                                start=(cc == 0), stop=(cc == 5))
                        nc.vector.tensor_scalar_add(
                            qkT[fc][:, 512 * t4:512 * (t4 + 1)], pq, bqk_sb[pos])

                def v_chunk(tt):
                    pv = pog.tile([128, GW], f32, name="po")
                    for cc in range(6):
                        nc.tensor.matmul(
                            pv, xt_sb[cc][:, 128 * tt:128 * (tt + 1)], wv_sb[cc],
                            start=(cc == 0), stop=(cc == 5))
                    v3 = v1[tt]
                    nc.vector.tensor_add(
                        v3[:, :, 0:64],
                        pv.rearrange("p (h e) -> p h e", e=64),
                        bv_sb.rearrange("p (h e) -> p h e", e=64))
                    nc.gpsimd.memset(v3[:, :, 64:128], 1.0)

                def proj_range(tt_lo, tt_hi):
                    for tt in range(tt_lo, tt_hi):
                        o_sb = ob.tile([128, C], f32, name="o")
                        for nh in range(2):
                            pp = pog.tile([128, GW], f32, name="po")
                            for fc in range(3):
                                nc.tensor.matmul(
                                    pp, yT[fc][:, 128 * tt:128 * (tt + 1)],
                                    wp_sb[fc][:, GW * nh:GW * (nh + 1)],
                                    start=(fc == 0), stop=(fc == 2))
                            nc.vector.tensor_copy(o_sb[:, GW * nh:GW * (nh + 1)], pp)
                        nc.sync.dma_start(
                            out=out[128 * tt:128 * (tt + 1), :], in_=o_sb)

                def att_gen(h, icp):
                    r0 = 64 * (h % 2)
                    qh = qkT[h // 2][r0:r0 + 64, :]
                    kh = qkT[3 + h // 2][r0:r0 + 64, :]
                    i_lo = 1024 * icp
                    po2 = [pog.tile([128, 512], f32, name="po") for _ in range(2)]
                    for jt in range(8 * icp + 8):
                        j0 = 128 * jt
                        vs = max(j0 - i_lo, 0)
                        ps_t = psp.tile([128, 1024], f32, name="ps")
                        if vs < 512:
                            nc.tensor.matmul(
                                ps_t[:, vs:512], kh[:, j0:j0 + 128],
                                qh[:, i_lo + vs:i_lo + 512],
                                start=True, stop=True)
                            nc.tensor.matmul(
                                ps_t[:, 512:1024], kh[:, j0:j0 + 128],
                                qh[:, i_lo + 512:i_lo + 1024],
                                start=True, stop=True)
                        else:
                            nc.tensor.matmul(
                                ps_t[:, vs:1024], kh[:, j0:j0 + 128],
                                qh[:, i_lo + vs:i_lo + 1024],
                                start=True, stop=True)
                        pt_t = ptp.tile([128, 1024], IN_DT, name="pt")
                        nc.scalar.activation(
                            pt_t[:, vs:1024], ps_t[:, vs:1024], Exp, scale=SCALE)
                        if j0 >= i_lo:
                            # triangular mask on the diagonal block:
                            # keep where (i - j) = f - p >= 0, else 0
                            nc.gpsimd.affine_select(
                                out=pt_t[:, vs:vs + 128], in_=pt_t[:, vs:vs + 128],
                                compare_op=mybir.AluOpType.is_ge, fill=0.0,
                                base=0, pattern=[[1, 128]], channel_multiplier=-1)
                        for half in range(2):
                            hi = 512 * (half + 1)
                            stop_jt = 8 * icp + 4 * half + 3
                            if vs < hi:
                                rl = max(vs, 512 * half)
                                nc.tensor.matmul(
                                    po2[half][:, rl - 512 * half:512],
                                    v1[jt][:, h, :], pt_t[:, rl:hi],
                                    start=(jt == 0), stop=(jt == stop_jt))
                            if jt == stop_jt:
                                # normalize this half as soon as its
                                # accumulation closes: po rows 64:128 hold
                                # the denominator replicated across 64
                                # partitions (ones block) -> recip + mul.
                                bc_sb = nrm.tile([64, 512], f32, name="bc")
                                nc.vector.reciprocal(bc_sb, po2[half][64:128, :])
                                nc.vector.tensor_mul(
                                    yT[h // 2][r0:r0 + 64,
                                               i_lo + 512 * half:
                                               i_lo + 512 * (half + 1)],
                                    po2[half][0:64, :], bc_sb)
                        yield

                def att(h, icp):
                    for _ in att_gen(h, icp):
                        pass

                _S = object()

                def att_pair(h):
                    # interleave the two i-range streams 1:2 so PE always has
                    # an independent QK to issue while ACT drains the other
                    g0, g1 = att_gen(h, 0), att_gen(h, 1)
                    done0 = done1 = False
                    k = 0
                    while not (done0 and done1):
                        if not done1:
                            done1 = next(g1, _S) is _S
                        if k % 2 == 1 and not done0:
                            done0 = next(g0, _S) is _S
                        k += 1

                qk_chunk(0)
                qk_chunk(3)
                for tt in range(8):
                    v_chunk(tt)
                att(0, 0)
                for tt in range(8, 16):
                    v_chunk(tt)
                att(0, 1)
                qk_chunk(1)
                att(1, 0)
                qk_chunk(4)
                att(1, 1)
                qk_chunk(2)
                att(2, 0)
                qk_chunk(5)
                att(2, 1)
                att(3, 0)
                att(4, 0)
                att(5, 0)
                att(3, 1)
                proj_range(0, 3)
                att(4, 1)
                proj_range(3, 6)
                g = att_gen(5, 1)
                k = 0
                for _ in g:
                    # ride remaining projection tiles inside the last head's
                    # jt loop as their yT ranges become ready
                    if k == 5:
                        proj_range(6, 7)
                    elif k == 10:
                        proj_range(7, 8)
                    elif k == 12:
                        proj_range(8, 10)
                    elif k == 14:
                        proj_range(10, 12)
                    k += 1
                proj_range(12, 16)


_CACHE = {}


def _build(n_reps=1):
    key = ("nc", n_reps)
    if key in _CACHE:
        return _CACHE[key]
    nc = bacc.Bacc("TRN2", target_bir_lowering=False, debug=False)
    xt = nc.dram_tensor("xt", [C, T], IN_DT, kind="ExternalInput")
    w_qk = nc.dram_tensor("w_qk", [C, 2 * GW], IN_DT, kind="ExternalInput")
    w_v = nc.dram_tensor("w_v", [C, GW], IN_DT, kind="ExternalInput")
    b_qk = nc.dram_tensor("b_qk", [2 * GW], f32, kind="ExternalInput")
    b_v = nc.dram_tensor("b_v", [GW], f32, kind="ExternalInput")
    w_p = nc.dram_tensor("w_p", [GW, C], IN_DT, kind="ExternalInput")
    out = nc.dram_tensor("out", [T, C], f32, kind="ExternalOutput")
    with tile.TileContext(nc) as tc:
        _emit(tc, xt[:, :], w_qk[:, :], w_v[:, :], b_qk[:], b_v[:], w_p[:, :],
              out[:, :], n_reps=n_reps)
    nc.compile()
    _CACHE[key] = nc
    return nc


def make_in_maps(x, w_attn, b_attn, w_proj):
    import ml_dtypes
    nbf16 = ml_dtypes.bfloat16
    x = np.ascontiguousarray(np.asarray(x, dtype=np.float32)).astype(nbf16)
    w_attn = np.asarray(w_attn, dtype=np.float32).astype(nbf16)
    b_attn = np.asarray(b_attn, dtype=np.float32)
    w_proj = np.asarray(w_proj, dtype=np.float32).astype(nbf16)
    in_maps = []
    for c in range(8):
        b, s = c // 2, c % 2
        q = slice(GW * s, GW * (s + 1))
        k = slice(C + GW * s, C + GW * (s + 1))
        v = slice(2 * C + GW * s, 2 * C + GW * (s + 1))
        wqk_full = np.concatenate([w_attn[:, q], w_attn[:, k]], axis=1)
        bqk_full = np.concatenate([b_attn[q], b_attn[k]])
        wqk_ord = np.concatenate(
            [wqk_full[:, 128 * fc:128 * (fc + 1)] for fc in FC_ORDER], axis=1)
        bqk_ord = np.concatenate(
            [bqk_full[128 * fc:128 * (fc + 1)] for fc in FC_ORDER])
        in_maps.append({
            "xt": np.ascontiguousarray(x[b].T),
            "w_qk": np.ascontiguousarray(wqk_ord),
            "w_v": np.ascontiguousarray(w_attn[:, v]),
            "b_qk": np.ascontiguousarray(bqk_ord),
            "b_v": np.ascontiguousarray(b_attn[v]),
            "w_p": np.ascontiguousarray(w_proj[GW * s:GW * (s + 1), :]),
        })
    return in_maps


def combine_outputs(results, b_proj):
    b_proj = np.asarray(b_proj, dtype=np.float32)
    outs = [results[c]["out"] for c in range(8)]
    y = np.stack([outs[2 * b] + outs[2 * b + 1] for b in range(B)])
    return (y + b_proj[None, None, :]).astype(np.float32)


def kernel(x, w_attn, b_attn, w_proj, b_proj, last_k_no_attend=0, window_size=0):
    # last_k_no_attend / window_size are 0 in this problem (no-op branch).
    nc = _build()
    in_maps = make_in_maps(x, w_attn, b_attn, w_proj)
    res = run_bass_kernel_spmd(nc, in_maps, list(range(8)))
    return combine_outputs(res.results, b_proj)


# revision 37
# speedup vs baseline: 1.1370x; 1.1370x over previous
"""Causal self-attention (B=4, T=2048, C=768, H=12) on 8 TRN2 NeuronCores.

Sharding: data-parallel over batch (4) x tensor-parallel over heads (2 groups
of 6).  Core c handles batch c//2, head-group c%2.  Each core computes its
QKV projection slice, causal flash-attention for its 6 heads, and a partial
output projection; the host sums the two head-group partials per batch and
adds b_proj.

On-device layout (per core):
  xt    [C, T]   x[b] transposed (host-side) so features sit on partitions.
  qkT   [2*384, T] q^T | k^T feature-major, computed as w_qk^T-chunk @ x^T
        (w_qk columns are pre-shuffled host-side into FC_ORDER so the
        blocks needed first arrive in the leading DMA).
  v1    [T, 6, 128] v natural (token-major) with a 64-wide ones block per
        head: the PV matmul (time is moving-dim bound, the extra stationary
        columns are free) then emits the softmax denominator already
        replicated across 64 partitions, so normalization is just
        reciprocal + multiply on VectorE -- no cross-partition broadcast.
  S^T   computed per 128-row j-tile as k @ q^T so the softmax reduction
        (over j) lands on the partition axis and is folded into the PV
        matmul; exp() runs on ScalarE straight out of PSUM; the causal
        triangle is an affine_select on GpSimd; fully-masked columns are
        never computed or exp'ed.
  yT    [384, T] normalized attention output, feature-major, feeding the
        output projection with w_p natural.
All tensors stream as bf16 (fp32 PSUM accumulation); emission order
hand-interleaves the QKV projection, attention and output projection so
TensorE and ScalarE (exp is ~115us of intrinsic work) overlap; measured
~190-230us/core on HW, output rel err ~3e-3 vs the fp32 reference.
"""
import sys

try:
    import concourse  # noqa: F401
except ImportError:
    sys.path.insert(0, "/opt/trn_rl_repo")

import numpy as np
import concourse.bacc as bacc
import concourse.mybir as mybir
import concourse.tile as tile
from concourse.bass_utils import run_bass_kernel_spmd

f32 = mybir.dt.float32
bf16 = mybir.dt.bfloat16
IN_DT = bf16     # streamed inputs (xt, w_qk, w_v, w_p) + yT
Exp = mybir.ActivationFunctionType.Exp

B, T, C, H = 4, 2048, 768, 12
FC_ORDER = [0, 3, 1, 4, 2, 5]   # host lays w_qk/b_qk columns out in this
FC_POS = {fc: i for i, fc in enumerate(FC_ORDER)}  # feature-chunk order
HD = 64          # head dim
GW = 384         # head-group width (6 heads)
NCC = C // 128   # 6 contraction chunks
SCALE = HD ** -0.5


def _emit(tc, xt, w_qk, w_v, b_qk, b_v, w_p, out, n_reps=1):
    nc = tc.nc

    with tc.tile_pool(name="const", bufs=1) as const, \
         tc.tile_pool(name="qkv", bufs=1) as qkv, \
         tc.tile_pool(name="psp", bufs=2, space="PSUM") as psp, \
         tc.tile_pool(name="pog", bufs=4, space="PSUM") as pog, \
         tc.tile_pool(name="ptp", bufs=16) as ptp, \
         tc.tile_pool(name="nrm", bufs=6) as nrm, \
         tc.tile_pool(name="ob", bufs=5) as ob:
        # ---- constants (tiles only; DMAs issued after the phase-1 bulk
        # loads so their queue triggers don't delay the first matmuls)
        bqk_all = const.tile([128, 6], f32, name="bqk")
        bqk_sb = [bqk_all[:, fc:fc + 1] for fc in range(6)]
        bv_sb = const.tile([128, GW], f32, name="bv")
        ones6 = const.tile([128, 6], f32, name="ones6")
        nc.vector.memset(ones6, 1.0)
        wp_all = const.tile([128, 3, C], IN_DT, name="wp")
        wp_sb = [wp_all[:, fc, :] for fc in range(3)]

        def load_consts():
            nc.sync.dma_start(
                out=bqk_all, in_=b_qk[:].rearrange("(fc p) -> p fc", p=128))
            nc.sync.dma_start(
                out=bv_sb,
                in_=b_v[:][None, :].partition_broadcast(128).opt(keep_dims={0}))
            nc.sync.dma_start(
                out=wp_all, in_=w_p[:, :].rearrange("(fc p) n -> p fc n", p=128))

        # ---- persistent per-rep tensors
        qkT = [qkv.tile([128, T], IN_DT, name=f"qkT{fc}") for fc in range(6)]
        v1 = [qkv.tile([128, 6, 128], IN_DT, name=f"v1_{tt}") for tt in range(16)]
        yT = [qkv.tile([128, T], IN_DT, name=f"yT{fc}") for fc in range(3)]

        for _ in range(n_reps):
            # ================= phase 1: QKV projection =================
            # pog slots are shared with attention PV accumulators and the
            # projection so phases overlap without a PSUM release barrier.
            with tc.tile_pool(name="xw", bufs=1) as xw:
                wqk_all = xw.tile([128, 6, 2 * GW], IN_DT, name="wqk")
                nc.sync.dma_start(
                    out=wqk_all[:, :, 0:256],
                    in_=w_qk[:, 0:256].rearrange("(cc p) f -> p cc f", p=128))
                wqk_sb = [wqk_all[:, cc, :] for cc in range(6)]
                xt_sb = [xw.tile([128, T], IN_DT, name=f"xt{cc}") for cc in range(6)]
                for cc in range(6):
                    nc.scalar.dma_start(out=xt_sb[cc][:, 0:1024],
                                        in_=xt[128 * cc:128 * (cc + 1), 0:1024])
                nc.sync.dma_start(
                    out=wqk_all[:, :, 256:2 * GW],
                    in_=w_qk[:, 256:2 * GW].rearrange("(cc p) f -> p cc f", p=128))
                for cc in range(6):
                    nc.scalar.dma_start(out=xt_sb[cc][:, 1024:T],
                                        in_=xt[128 * cc:128 * (cc + 1), 1024:T])
                wv_all = xw.tile([128, 6, GW], IN_DT, name="wv")
                nc.sync.dma_start(
                    out=wv_all, in_=w_v[:, :].rearrange("(cc p) f -> p cc f", p=128))
                wv_sb = [wv_all[:, cc, :] for cc in range(6)]
                load_consts()

                def qk_chunk(fc):
                    pos = FC_POS[fc]
                    for t4 in range(4):
                        pq = pog.tile([128, 512], f32, name="po")
                        for cc in range(6):
                            nc.tensor.matmul(
                                pq, wqk_sb[cc][:, 128 * pos:128 * (pos + 1)],
                                xt_sb[cc][:, 512 * t4:512 * (t4 + 1)],
                                start=(cc == 0), stop=(cc == 5))
                        nc.vector.tensor_scalar_add(
                            qkT[fc][:, 512 * t4:512 * (t4 + 1)], pq, bqk_sb[pos])

                def v_chunk(tt):
                    pv = pog.tile([128, GW], f32, name="po")
                    for cc in range(6):
                        nc.tensor.matmul(
                            pv, xt_sb[cc][:, 128 * tt:128 * (tt + 1)], wv_sb[cc],
                            start=(cc == 0), stop=(cc == 5))
                    v3 = v1[tt]
                    nc.vector.tensor_add(
                        v3[:, :, 0:64],
                        pv.rearrange("p (h e) -> p h e", e=64),
                        bv_sb.rearrange("p (h e) -> p h e", e=64))
                    nc.gpsimd.memset(v3[:, :, 64:128], 1.0)

                def proj_range(tt_lo, tt_hi):
                    for tt in range(tt_lo, tt_hi):
                        o_sb = ob.tile([128, C], f32, name="o")
                        for nh in range(2):
                            pp = pog.tile([128, GW], f32, name="po")
                            for fc in range(3):
                                nc.tensor.matmul(
                                    pp, yT[fc][:, 128 * tt:128 * (tt + 1)],
                                    wp_sb[fc][:, GW * nh:GW * (nh + 1)],
                                    start=(fc == 0), stop=(fc == 2))
                            nc.vector.tensor_copy(o_sb[:, GW * nh:GW * (nh + 1)], pp)
                        nc.sync.dma_start(
                            out=out[128 * tt:128 * (tt + 1), :], in_=o_sb)

                def att_gen(h, icp):
                    r0 = 64 * (h % 2)
                    qh = qkT[h // 2][r0:r0 + 64, :]
                    kh = qkT[3 + h // 2][r0:r0 + 64, :]
                    i_lo = 1024 * icp
                    po2 = [pog.tile([128, 512], f32, name="po") for _ in range(2)]
                    for jt in range(8 * icp + 8):
                        j0 = 128 * jt
                        vs = max(j0 - i_lo, 0)
                        ps_t = psp.tile([128, 1024], f32, name="ps")
                        if vs < 512:
                            nc.tensor.matmul(
                                ps_t[:, vs:512], kh[:, j0:j0 + 128],
                                qh[:, i_lo + vs:i_lo + 512],
                                start=True, stop=True)
                            nc.tensor.matmul(
                                ps_t[:, 512:1024], kh[:, j0:j0 + 128],
                                qh[:, i_lo + 512:i_lo + 1024],
                                start=True, stop=True)
                        else:
                            nc.tensor.matmul(
                                ps_t[:, vs:1024], kh[:, j0:j0 + 128],
                                qh[:, i_lo + vs:i_lo + 1024],
                                start=True, stop=True)
                        pt_t = ptp.tile([128, 1024], IN_DT, name="pt")
                        nc.scalar.activation(
                            pt_t[:, vs:1024], ps_t[:, vs:1024], Exp, scale=SCALE)
                        if j0 >= i_lo:
                            # triangular mask on the diagonal block:
                            # keep where (i - j) = f - p >= 0, else 0
                            nc.gpsimd.affine_select(
                                out=pt_t[:, vs:vs + 128], in_=pt_t[:, vs:vs + 128],
                                compare_op=mybir.AluOpType.is_ge, fill=0.0,
                                base=0, pattern=[[1, 128]], channel_multiplier=-1)
                        for half in range(2):
                            hi = 512 * (half + 1)
                            stop_jt = 8 * icp + 4 * half + 3
                            if vs < hi:
                                rl = max(vs, 512 * half)
                                nc.tensor.matmul(
                                    po2[half][:, rl - 512 * half:512],
                                    v1[jt][:, h, :], pt_t[:, rl:hi],
                                    start=(jt == 0), stop=(jt == stop_jt))
                            if jt == stop_jt:
                                # normalize this half as soon as its
                                # accumulation closes: po rows 64:128 hold
                                # the denominator replicated across 64
                                # partitions (ones block) -> recip + mul.
                                bc_sb = nrm.tile([64, 512], f32, name="bc")
                                nc.vector.reciprocal(bc_sb, po2[half][64:128, :])
                                nc.vector.tensor_mul(
                                    yT[h // 2][r0:r0 + 64,
                                               i_lo + 512 * half:
                                               i_lo + 512 * (half + 1)],
                                    po2[half][0:64, :], bc_sb)
                        yield

                def att(h, icp):
                    for _ in att_gen(h, icp):
                        pass

                _S = object()

                def att_pair(h):
                    # interleave the two i-range streams 1:2 so PE always has
                    # an independent QK to issue while ACT drains the other
                    g0, g1 = att_gen(h, 0), att_gen(h, 1)
                    done0 = done1 = False
                    k = 0
                    while not (done0 and done1):
                        if not done1:
                            done1 = next(g1, _S) is _S
                        if k % 2 == 1 and not done0:
                            done0 = next(g0, _S) is _S
                        k += 1

                qk_chunk(0)
                qk_chunk(3)
                for tt in range(8):
                    v_chunk(tt)
                att(0, 0)
                for tt in range(8, 16):
                    v_chunk(tt)
                att(0, 1)
                qk_chunk(1)
                att(1, 0)
                qk_chunk(4)
                att(1, 1)
                qk_chunk(2)
                att(2, 0)
                qk_chunk(5)
                att(2, 1)
                att(3, 0)
                att(4, 0)
                att(5, 0)
                att(3, 1)
                proj_range(0, 3)
                att(4, 1)
                proj_range(3, 6)
                g = att_gen(5, 1)
                k = 0
                for _ in g:
                    # ride remaining projection tiles inside the last head's
                    # jt loop as their yT ranges become ready
                    if k == 5:
                        proj_range(6, 7)
                    elif k == 10:
                        proj_range(7, 8)
                    elif k == 12:
                        proj_range(8, 10)
                    elif k == 14:
                        proj_range(10, 12)
                    k += 1
                proj_range(12, 16)


_CACHE = {}


def _build(n_reps=1):
    key = ("nc", n_reps)
    if key in _CACHE:
        return _CACHE[key]
    nc = bacc.Bacc("TRN2", target_bir_lowering=False, debug=False)
    xt = nc.dram_tensor("xt", [C, T], IN_DT, kind="ExternalInput")
    w_qk = nc.dram_tensor("w_qk", [C, 2 * GW], IN_DT, kind="ExternalInput")
    w_v = nc.dram_tensor("w_v", [C, GW], IN_DT, kind="ExternalInput")
    b_qk = nc.dram_tensor("b_qk", [2 * GW], f32, kind="ExternalInput")
    b_v = nc.dram_tensor("b_v", [GW], f32, kind="ExternalInput")
    w_p = nc.dram_tensor("w_p", [GW, C], IN_DT, kind="ExternalInput")
    out = nc.dram_tensor("out", [T, C], f32, kind="ExternalOutput")
    with tile.TileContext(nc) as tc:
        _emit(tc, xt[:, :], w_qk[:, :], w_v[:, :], b_qk[:], b_v[:], w_p[:, :],
              out[:, :], n_reps=n_reps)
    nc.compile()
    _CACHE[key] = nc
    return nc


def make_in_maps(x, w_attn, b_attn, w_proj):
    import ml_dtypes
    nbf16 = ml_dtypes.bfloat16
    x = np.ascontiguousarray(np.asarray(x, dtype=np.float32)).astype(nbf16)
    w_attn = np.asarray(w_attn, dtype=np.float32).astype(nbf16)
    b_attn = np.asarray(b_attn, dtype=np.float32)
    w_proj = np.asarray(w_proj, dtype=np.float32).astype(nbf16)
    in_maps = []
    for c in range(8):
        b, s = c // 2, c % 2
        q = slice(GW * s, GW * (s + 1))
        k = slice(C + GW * s, C + GW * (s + 1))
        v = slice(2 * C + GW * s, 2 * C + GW * (s + 1))
        wqk_full = np.concatenate([w_attn[:, q], w_attn[:, k]], axis=1)
        bqk_full = np.concatenate([b_attn[q], b_attn[k]])
        wqk_ord = np.concatenate(
            [wqk_full[:, 128 * fc:128 * (fc + 1)] for fc in FC_ORDER], axis=1)
        bqk_ord = np.concatenate(
            [bqk_full[128 * fc:128 * (fc + 1)] for fc in FC_ORDER])
        in_maps.append({
            "xt": np.ascontiguousarray(x[b].T),
            "w_qk": np.ascontiguousarray(wqk_ord),
            "w_v": np.ascontiguousarray(w_attn[:, v]),
            "b_qk": np.ascontiguousarray(bqk_ord),
            "b_v": np.ascontiguousarray(b_attn[v]),
            "w_p": np.ascontiguousarray(w_proj[GW * s:GW * (s + 1), :]),
        })
    return in_maps


def combine_outputs(results, b_proj):
    b_proj = np.asarray(b_proj, dtype=np.float32)
    outs = [results[c]["out"] for c in range(8)]
    y = np.stack([outs[2 * b] + outs[2 * b + 1] for b in range(B)])
    return (y + b_proj[None, None, :]).astype(np.float32)


def kernel(x, w_attn, b_attn, w_proj, b_proj, last_k_no_attend=0, window_size=0):
    # last_k_no_attend / window_size are 0 in this problem (no-op branch).
    nc = _build()
    in_maps = make_in_maps(x, w_attn, b_attn, w_proj)
    res = run_bass_kernel_spmd(nc, in_maps, list(range(8)))
    return combine_outputs(res.results, b_proj)


# revision 39
# speedup vs baseline: 1.6602x; 1.4601x over previous
"""Causal self-attention (B=4, T=2048, C=768, H=12) on 8 TRN2 NeuronCores.

Sharding: data-parallel over batch (4) x tensor-parallel over heads (2 groups
of 6).  Core c handles batch c//2, head-group c%2.  Each core computes its
QKV projection slice, causal flash-attention for its 6 heads, and a partial
output projection; the host sums the two head-group partials per batch and
adds b_proj.

On-device layout (per core):
  xt    [C, T]   x[b] transposed (host-side) so features sit on partitions.
  qkT   [2*384, T] q^T | k^T feature-major, computed as w_qk^T-chunk @ x^T
        (w_qk columns are pre-shuffled host-side into FC_ORDER so the
        blocks needed first arrive in the leading DMA).
  v1    [T, 6, 128] v natural (token-major) with a 64-wide ones block per
        head: the PV matmul (time is moving-dim bound, the extra stationary
        columns are free) then emits the softmax denominator already
        replicated across 64 partitions, so normalization is just
        reciprocal + multiply on VectorE -- no cross-partition broadcast.
  S^T   computed per 128-row j-tile as k @ q^T so the softmax reduction
        (over j) lands on the partition axis and is folded into the PV
        matmul; exp() runs on ScalarE straight out of PSUM; the causal
        triangle is an affine_select on GpSimd; fully-masked columns are
        never computed or exp'ed.
  yT    [384, T] normalized attention output, feature-major, feeding the
        output projection with w_p natural.
All tensors stream as bf16 (fp32 PSUM accumulation); emission order
hand-interleaves the QKV projection, attention and output projection so
TensorE (~154us busy) and ScalarE (~115us of intrinsic exp work) overlap,
with per-half normalization emitted at each accumulator's closing j-tile
and the tail projection riding inside the last head's jt loop.
Cost-model 199us; measured ~200-230us/core on HW; rel err ~3e-3 vs the
fp32 reference.
"""
import sys

try:
    import concourse  # noqa: F401
except ImportError:
    sys.path.insert(0, "/opt/trn_rl_repo")

import numpy as np
import concourse.bacc as bacc
import concourse.mybir as mybir
import concourse.tile as tile
from concourse.bass_utils import run_bass_kernel_spmd

f32 = mybir.dt.float32
bf16 = mybir.dt.bfloat16
IN_DT = bf16     # streamed inputs (xt, w_qk, w_v, w_p) + yT
Exp = mybir.ActivationFunctionType.Exp

B, T, C, H = 4, 2048, 768, 12
FC_ORDER = [0, 3, 1, 4, 2, 5]   # host lays w_qk/b_qk columns out in this
FC_POS = {fc: i for i, fc in enumerate(FC_ORDER)}  # feature-chunk order
HD = 64          # head dim
GW = 384         # head-group width (6 heads)
NCC = C // 128   # 6 contraction chunks
SCALE = HD ** -0.5


def _emit(tc, xt, w_qk, w_v, b_qk, b_v, w_p, out, n_reps=1):
    nc = tc.nc

    with tc.tile_pool(name="const", bufs=1) as const, \
         tc.tile_pool(name="qkv", bufs=1) as qkv, \
         tc.tile_pool(name="psp", bufs=2, space="PSUM") as psp, \
         tc.tile_pool(name="pog", bufs=4, space="PSUM") as pog, \
         tc.tile_pool(name="ptp", bufs=20) as ptp, \
         tc.tile_pool(name="nrm", bufs=8) as nrm, \
         tc.tile_pool(name="ob", bufs=5) as ob:
        # ---- constants (tiles only; DMAs issued after the phase-1 bulk
        # loads so their queue triggers don't delay the first matmuls)
        bqk_all = const.tile([128, 6], f32, name="bqk")
        bqk_sb = [bqk_all[:, fc:fc + 1] for fc in range(6)]
        bv_sb = const.tile([128, GW], f32, name="bv")
        ones6 = const.tile([128, 6], f32, name="ones6")
        nc.vector.memset(ones6, 1.0)
        wp_all = const.tile([128, 3, C], IN_DT, name="wp")
        wp_sb = [wp_all[:, fc, :] for fc in range(3)]

        def load_consts():
            nc.sync.dma_start(
                out=bqk_all, in_=b_qk[:].rearrange("(fc p) -> p fc", p=128))
            nc.sync.dma_start(
                out=bv_sb,
                in_=b_v[:][None, :].partition_broadcast(128).opt(keep_dims={0}))
            nc.sync.dma_start(
                out=wp_all, in_=w_p[:, :].rearrange("(fc p) n -> p fc n", p=128))

        # ---- persistent per-rep tensors
        qkT = [qkv.tile([128, T], IN_DT, name=f"qkT{fc}") for fc in range(6)]
        v1 = [qkv.tile([128, 6, 128], IN_DT, name=f"v1_{tt}") for tt in range(16)]
        yT = [qkv.tile([128, T], IN_DT, name=f"yT{fc}") for fc in range(3)]

        for _ in range(n_reps):
            # ================= phase 1: QKV projection =================
            # pog slots are shared with attention PV accumulators and the
            # projection so phases overlap without a PSUM release barrier.
            with tc.tile_pool(name="xw", bufs=1) as xw:
                wqk_all = xw.tile([128, 6, 2 * GW], IN_DT, name="wqk")
                nc.sync.dma_start(
                    out=wqk_all[:, :, 0:256],
                    in_=w_qk[:, 0:256].rearrange("(cc p) f -> p cc f", p=128))
                wqk_sb = [wqk_all[:, cc, :] for cc in range(6)]
                xt_sb = [xw.tile([128, T], IN_DT, name=f"xt{cc}") for cc in range(6)]
                for cc in range(6):
                    nc.scalar.dma_start(out=xt_sb[cc][:, 0:1024],
                                        in_=xt[128 * cc:128 * (cc + 1), 0:1024])
                nc.sync.dma_start(
                    out=wqk_all[:, :, 256:2 * GW],
                    in_=w_qk[:, 256:2 * GW].rearrange("(cc p) f -> p cc f", p=128))
                for cc in range(6):
                    nc.scalar.dma_start(out=xt_sb[cc][:, 1024:T],
                                        in_=xt[128 * cc:128 * (cc + 1), 1024:T])
                wv_all = xw.tile([128, 6, GW], IN_DT, name="wv")
                nc.sync.dma_start(
                    out=wv_all, in_=w_v[:, :].rearrange("(cc p) f -> p cc f", p=128))
                wv_sb = [wv_all[:, cc, :] for cc in range(6)]
                load_consts()

                def qk_chunk(fc):
                    pos = FC_POS[fc]
                    for t4 in range(4):
                        pq = pog.tile([128, 512], f32, name="po")
                        for cc in range(6):
                            nc.tensor.matmul(
                                pq, wqk_sb[cc][:, 128 * pos:128 * (pos + 1)],
                                xt_sb[cc][:, 512 * t4:512 * (t4 + 1)],
                                start=(cc == 0), stop=(cc == 5))
                        nc.vector.tensor_scalar_add(
                            qkT[fc][:, 512 * t4:512 * (t4 + 1)], pq, bqk_sb[pos])

                def v_chunk(tt):
                    pv = pog.tile([128, GW], f32, name="po")
                    for cc in range(6):
                        nc.tensor.matmul(
                            pv, xt_sb[cc][:, 128 * tt:128 * (tt + 1)], wv_sb[cc],
                            start=(cc == 0), stop=(cc == 5))
                    v3 = v1[tt]
                    nc.vector.tensor_add(
                        v3[:, :, 0:64],
                        pv.rearrange("p (h e) -> p h e", e=64),
                        bv_sb.rearrange("p (h e) -> p h e", e=64))
                    nc.gpsimd.memset(v3[:, :, 64:128], 1.0)

                def proj_range(tt_lo, tt_hi):
                    for tt in range(tt_lo, tt_hi):
                        o_sb = ob.tile([128, C], f32, name="o")
                        for nh in range(2):
                            pp = pog.tile([128, GW], f32, name="po")
                            for fc in range(3):
                                nc.tensor.matmul(
                                    pp, yT[fc][:, 128 * tt:128 * (tt + 1)],
                                    wp_sb[fc][:, GW * nh:GW * (nh + 1)],
                                    start=(fc == 0), stop=(fc == 2))
                            nc.vector.tensor_copy(o_sb[:, GW * nh:GW * (nh + 1)], pp)
                        nc.sync.dma_start(
                            out=out[128 * tt:128 * (tt + 1), :], in_=o_sb)

                def att_gen(h, icp):
                    r0 = 64 * (h % 2)
                    qh = qkT[h // 2][r0:r0 + 64, :]
                    kh = qkT[3 + h // 2][r0:r0 + 64, :]
                    i_lo = 1024 * icp
                    po2 = [pog.tile([128, 512], f32, name="po") for _ in range(2)]
                    for jt in range(8 * icp + 8):
                        j0 = 128 * jt
                        vs = max(j0 - i_lo, 0)
                        ps_t = psp.tile([128, 1024], f32, name="ps")
                        if vs < 512:
                            nc.tensor.matmul(
                                ps_t[:, vs:512], kh[:, j0:j0 + 128],
                                qh[:, i_lo + vs:i_lo + 512],
                                start=True, stop=True)
                            nc.tensor.matmul(
                                ps_t[:, 512:1024], kh[:, j0:j0 + 128],
                                qh[:, i_lo + 512:i_lo + 1024],
                                start=True, stop=True)
                        else:
                            nc.tensor.matmul(
                                ps_t[:, vs:1024], kh[:, j0:j0 + 128],
                                qh[:, i_lo + vs:i_lo + 1024],
                                start=True, stop=True)
                        pt_t = ptp.tile([128, 1024], IN_DT, name="pt")
                        nc.scalar.activation(
                            pt_t[:, vs:1024], ps_t[:, vs:1024], Exp, scale=SCALE)
                        if j0 >= i_lo:
                            # triangular mask on the diagonal block:
                            # keep where (i - j) = f - p >= 0, else 0
                            nc.gpsimd.affine_select(
                                out=pt_t[:, vs:vs + 128], in_=pt_t[:, vs:vs + 128],
                                compare_op=mybir.AluOpType.is_ge, fill=0.0,
                                base=0, pattern=[[1, 128]], channel_multiplier=-1)
                        for half in range(2):
                            hi = 512 * (half + 1)
                            stop_jt = 8 * icp + 4 * half + 3
                            if vs < hi:
                                rl = max(vs, 512 * half)
                                nc.tensor.matmul(
                                    po2[half][:, rl - 512 * half:512],
                                    v1[jt][:, h, :], pt_t[:, rl:hi],
                                    start=(jt == 0), stop=(jt == stop_jt))
                            if jt == stop_jt:
                                # normalize this half as soon as its
                                # accumulation closes: po rows 64:128 hold
                                # the denominator replicated across 64
                                # partitions (ones block) -> recip + mul.
                                bc_sb = nrm.tile([64, 512], f32, name="bc")
                                nc.vector.reciprocal(bc_sb, po2[half][64:128, :])
                                nc.vector.tensor_mul(
                                    yT[h // 2][r0:r0 + 64,
                                               i_lo + 512 * half:
                                               i_lo + 512 * (half + 1)],
                                    po2[half][0:64, :], bc_sb)
                        yield

                def att(h, icp):
                    for _ in att_gen(h, icp):
                        pass

                _S = object()

                def att_pair(h):
                    # interleave the two i-range streams 1:2 so PE always has
                    # an independent QK to issue while ACT drains the other
                    g0, g1 = att_gen(h, 0), att_gen(h, 1)
                    done0 = done1 = False
                    k = 0
                    while not (done0 and done1):
                        if not done1:
                            done1 = next(g1, _S) is _S
                        if k % 2 == 1 and not done0:
                            done0 = next(g0, _S) is _S
                        k += 1

                qk_chunk(0)
                qk_chunk(3)
                for tt in range(8):
                    v_chunk(tt)
                att(0, 0)
                for tt in range(8, 16):
                    v_chunk(tt)
                att(0, 1)
                qk_chunk(1)
                att(1, 0)
                qk_chunk(4)
                att(1, 1)
                qk_chunk(2)
                att(2, 0)
                qk_chunk(5)
                att(2, 1)
                att(3, 0)
                att(4, 0)
                att(5, 0)
                att(3, 1)
                proj_range(0, 3)
                att(4, 1)
                proj_range(3, 6)
                g = att_gen(5, 1)
                k = 0
                for _ in g:
                    # ride remaining projection tiles inside the last head's
                    # jt loop as their yT ranges become ready
                    if k == 5:
                        proj_range(6, 7)
                    elif k == 10:
                        proj_range(7, 8)
                    elif k == 12:
                        proj_range(8, 10)
                    elif k == 14:
                        proj_range(10, 12)
                    k += 1
                proj_range(12, 16)


_CACHE = {}


def _build(n_reps=1):
    key = ("nc", n_reps)
    if key in _CACHE:
        return _CACHE[key]
    nc = bacc.Bacc("TRN2", target_bir_lowering=False, debug=False)
    xt = nc.dram_tensor("xt", [C, T], IN_DT, kind="ExternalInput")
    w_qk = nc.dram_tensor("w_qk", [C, 2 * GW], IN_DT, kind="ExternalInput")
    w_v = nc.dram_tensor("w_v", [C, GW], IN_DT, kind="ExternalInput")
    b_qk = nc.dram_tensor("b_qk", [2 * GW], f32, kind="ExternalInput")
    b_v = nc.dram_tensor("b_v", [GW], f32, kind="ExternalInput")
    w_p = nc.dram_tensor("w_p", [GW, C], IN_DT, kind="ExternalInput")
    out = nc.dram_tensor("out", [T, C], f32, kind="ExternalOutput")
    with tile.TileContext(nc) as tc:
        _emit(tc, xt[:, :], w_qk[:, :], w_v[:, :], b_qk[:], b_v[:], w_p[:, :],
              out[:, :], n_reps=n_reps)
    nc.compile()
    _CACHE[key] = nc
    return nc


def make_in_maps(x, w_attn, b_attn, w_proj):
    import ml_dtypes
    nbf16 = ml_dtypes.bfloat16
    x = np.ascontiguousarray(np.asarray(x, dtype=np.float32)).astype(nbf16)
    w_attn = np.asarray(w_attn, dtype=np.float32).astype(nbf16)
    b_attn = np.asarray(b_attn, dtype=np.float32)
    w_proj = np.asarray(w_proj, dtype=np.float32).astype(nbf16)
    in_maps = []
    for c in range(8):
        b, s = c // 2, c % 2
        q = slice(GW * s, GW * (s + 1))
        k = slice(C + GW * s, C + GW * (s + 1))
        v = slice(2 * C + GW * s, 2 * C + GW * (s + 1))
        wqk_full = np.concatenate([w_attn[:, q], w_attn[:, k]], axis=1)
        bqk_full = np.concatenate([b_attn[q], b_attn[k]])
        wqk_ord = np.concatenate(
            [wqk_full[:, 128 * fc:128 * (fc + 1)] for fc in FC_ORDER], axis=1)
        bqk_ord = np.concatenate(
            [bqk_full[128 * fc:128 * (fc + 1)] for fc in FC_ORDER])
        in_maps.append({
            "xt": np.ascontiguousarray(x[b].T),
            "w_qk": np.ascontiguousarray(wqk_ord),
            "w_v": np.ascontiguousarray(w_attn[:, v]),
            "b_qk": np.ascontiguousarray(bqk_ord),
            "b_v": np.ascontiguousarray(b_attn[v]),
            "w_p": np.ascontiguousarray(w_proj[GW * s:GW * (s + 1), :]),
        })
    return in_maps


def combine_outputs(results, b_proj):
    b_proj = np.asarray(b_proj, dtype=np.float32)
    outs = [results[c]["out"] for c in range(8)]
    y = np.stack([outs[2 * b] + outs[2 * b + 1] for b in range(B)])
    return (y + b_proj[None, None, :]).astype(np.float32)


def kernel(x, w_attn, b_attn, w_proj, b_proj, last_k_no_attend=0, window_size=0):
    # last_k_no_attend / window_size are 0 in this problem (no-op branch).
    nc = _build()
    in_maps = make_in_maps(x, w_attn, b_attn, w_proj)
    res = run_bass_kernel_spmd(nc, in_maps, list(range(8)))
    return combine_outputs(res.results, b_proj)
